# revision 1
# baseline (speedup 1.0000x reference)
"""Trainium2 Bass kernel for nn_AttentionModified (MQA-over-variants attention).

Strategy: data-parallel over B across 8 NeuronCores (no collectives — each
batch's output depends only on that batch's inputs).

Per-core pipeline (bf16 compute, f32 PSUM accumulation):
  - activations enter transposed via xbar transpose-DMA (contraction dim on
    partitions); weights pre-transposed on host
  - q^T/k^T/v^T via weight-stationary matmuls (K/V share one LDWEIGHTS across
    3 variants via parallel PSUM accumulation chains)
  - QK logits: broadcast-AP vector multiply (q repeated over variants), then a
    block-ones matmul reduces the 64-wide head groups -> s^T (12 heads, v*t)
  - softmax over V=8 without max-subtraction (logits ~N(0,0.3): exp can't
    overflow); Z via strided segmented reduce; normalization deferred to after
    the AV sum (reciprocal done token-major where it is 8x cheaper)
  - AV combine: broadcast-AP vector mults + pairwise adds
  - output projection; bias fused into PSUM eviction; output written
    transposed, host transposes back
Emission order software-pipelines the two 512-token halves so PE projection
work for half h+1 fills the gaps in the DVE-bound attention phase of half h.
"""
import sys

sys.path.insert(0, "/opt/trn_rl_repo")

import numpy as np
import ml_dtypes

import concourse.bass as bass
import concourse.mybir as mybir
import concourse.tile as tile
from concourse.bass_utils import run_bass_kernel_spmd

BF16 = mybir.dt.bfloat16
F32 = mybir.dt.float32
BF = ml_dtypes.bfloat16

V, B, N, C, H = 8, 8, 1024, 768, 12
HD = C // H  # 64
NK = C // 128  # 6 contraction chunks
HALF = 512
SCALE = HD ** -0.5


def _split_multi_waits(nc):
    """This container's walrus accepts only one sync-wait per instruction;
    hoist extra waits onto same-engine NoOps inserted just before."""
    for f in nc.m.functions:
        for bb in f.blocks:
            new = []
            for inst in bb.instructions:
                si = inst.sync_info
                waits = list(si.on_wait) if (si and si.on_wait) else []
                if len(waits) > 1:
                    for i, w in enumerate(waits[:-1]):
                        nop = mybir.InstNoOp(name=f"{inst.name}-wsplit{i}")
                        nop.engine = inst.engine
                        nop.sync_info = mybir.SyncInfo(on_wait=[w], on_update=[])
                        new.append(nop)
                    si.on_wait = [waits[-1]]
                new.append(inst)
            bb.instructions[:] = new


def _bc(a, dims):
    """Rebuild AP `a` with an explicit dim list (partition dim first)."""
    return bass.AP(tensor=a.tensor, offset=a.offset, ap=dims)


def build_kernel():
    nc = bass.Bass("TRN2", target_bir_lowering=False, debug=False, num_devices=8)

    xw = nc.dram_tensor("xw", [N, C], BF16, kind="ExternalInput").ap()
    vp = nc.dram_tensor("vp", [V, N, C], BF16, kind="ExternalInput").ap()
    wq = nc.dram_tensor("wq", [C, C], BF16, kind="ExternalInput").ap()
    wkk = nc.dram_tensor("wkk", [C, 128], BF16, kind="ExternalInput").ap()
    wv = nc.dram_tensor("wv", [C, HD], BF16, kind="ExternalInput").ap()
    wp = nc.dram_tensor("wp", [C, C], BF16, kind="ExternalInput").ap()
    bp = nc.dram_tensor("bp", [C, 1], F32, kind="ExternalInput").ap()
    ones = nc.dram_tensor("ones", [C, H], BF16, kind="ExternalInput").ap()
    ident = nc.dram_tensor("ident", [128, 128], BF16, kind="ExternalInput").ap()
    identf = nc.dram_tensor("identf", [12, 12], F32, kind="ExternalInput").ap()
    repl = nc.dram_tensor("repl", [12, C], BF16, kind="ExternalInput").ap()
    outt = nc.dram_tensor("outt", [C, N], F32, kind="ExternalOutput").ap()

    EXP = mybir.ActivationFunctionType.Exp
    IDENT = mybir.ActivationFunctionType.Identity

    with tile.TileContext(nc) as tc:
        with (
            tc.tile_pool(name="singles", bufs=1) as singles,
            tc.tile_pool(name="vtp", bufs=2) as vtp_pool,
            tc.tile_pool(name="acts", bufs=2) as acts,
            tc.tile_pool(name="acts1", bufs=1) as acts1,
            tc.tile_pool(name="tmp", bufs=2) as tmp_pool,
            tc.tile_pool(name="sm", bufs=2) as sm_pool,
            tc.tile_pool(name="av", bufs=1) as av_pool,
            tc.tile_pool(name="outp", bufs=1) as out_pool,
            tc.tile_pool(name="psmm", bufs=4, space="PSUM") as psum_mm,
            tc.tile_pool(name="psss", bufs=2, space="PSUM") as psum_s,
            tc.tile_pool(name="pstr", bufs=2, space="PSUM") as psum_tr,
        ):
            # ---- constants (emission order = sync-ring order: small K weight
            # first, then the first transposes, so attention can start early)
            wkk_sb = singles.tile([128, NK, 128], BF16)
            nc.sync.dma_start(out=wkk_sb[:], in_=wkk.rearrange("(j p) o -> p j o", p=128))
            ones_sb = singles.tile([128, NK, H], BF16)
            nc.sync.dma_start(out=ones_sb[:], in_=ones.rearrange("(j p) o -> p j o", p=128))
            id_sb = singles.tile([128, 128], BF16)
            nc.sync.dma_start(out=id_sb[:], in_=ident)
            idf_sb = singles.tile([12, 12], F32)
            nc.sync.dma_start(out=idf_sb[:], in_=identf)
            repl_sb = singles.tile([12, C], BF16)
            nc.sync.dma_start(out=repl_sb[:], in_=repl)
            wq_sb = singles.tile([128, NK, C], BF16)
            wv_sb = singles.tile([128, NK, HD], BF16)
            wp_sb = singles.tile([128, NK, C], BF16)
            bp_sb = singles.tile([128, NK], F32)
            xt_sb = singles.tile([128, NK, N], BF16)


            def emit_late_consts():
                nc.sync.dma_start(
                    out=xt_sb[:, :, HALF:N], in_=xw[HALF:N, :], transpose=True
                )
                nc.sync.dma_start(out=wv_sb[:], in_=wv.rearrange("(j p) o -> p j o", p=128))
                nc.sync.dma_start(out=wp_sb[:], in_=wp.rearrange("(j p) o -> p j o", p=128))
                nc.sync.dma_start(out=bp_sb[:], in_=bp.rearrange("(j p) 1 -> p j", p=128))

            def emit_transposes(h2, vpt=None, vs=None):
                T0 = h2 * HALF
                if vpt is None:
                    vpt = vtp_pool.tile([128, NK, V, HALF], BF16, tag="vpt", name="vpt")
                for v in (range(V) if vs is None else vs):
                    nc.sync.dma_start(
                        out=vpt[:, :, v, :],
                        in_=vp[v, T0 : T0 + HALF, :],
                        transpose=True,
                    )
                return vpt

            def emit_q_chunks(qt, h2, ms):
                T0 = h2 * HALF
                for m in ms:
                    psq = psum_mm.tile([128, HALF], F32, tag="mm", name="psq")
                    for k in range(NK):
                        nc.tensor.matmul(
                            psq[:],
                            lhsT=wq_sb[:, k, m * 128 : (m + 1) * 128],
                            rhs=xt_sb[:, k, T0 : T0 + HALF],
                            start=(k == 0),
                            stop=(k == NK - 1),
                        )
                    nc.scalar.copy(qt[:, m, :], psq[:])

            def emit_q(h2):
                qt = acts.tile([128, NK, HALF], BF16, tag="qt", name="qt")
                emit_q_chunks(qt, h2, range(NK))
                return qt

            def emit_k_group(kt, vpt, g0):
                for _ in (0,):
                    gn = 4
                    psks = [
                        psum_mm.tile([128, HALF], F32, tag="mm", name=f"psk{i}")
                        for i in range(gn)
                    ]
                    for k in range(NK):
                        for i in range(gn):
                            nc.tensor.matmul(
                                psks[i][:],
                                lhsT=wkk_sb[:, k, :],
                                rhs=vpt[:, k, g0 + i, :],
                                start=(k == 0),
                                stop=(k == NK - 1),
                            )
                    for i in range(gn):
                        nc.scalar.copy(kt[:, g0 + i, :], psks[i][:])

            def emit_k(h2, vpt):
                kt = acts.tile([128, V, HALF], BF16, tag="kt", name="kt")
                emit_k_group(kt, vpt, 0)
                emit_k_group(kt, vpt, 4)
                return kt

            def emit_v(h2, vpt):
                vt = acts.tile([64, V, HALF], BF16, name="vt")
                for g0 in (0, 4):
                    gn = 4
                    psvs = [
                        psum_mm.tile([64, HALF], F32, tag="mm", name=f"psv{i}")
                        for i in range(gn)
                    ]
                    for k in range(NK):
                        for i in range(gn):
                            nc.tensor.matmul(
                                psvs[i][:],
                                lhsT=wv_sb[:, k, :],
                                rhs=vpt[:, k, g0 + i, :],
                                start=(k == 0),
                                stop=(k == NK - 1),
                            )
                    for i in range(gn):
                        nc.scalar.copy(vt[:, g0 + i, :], psvs[i][:])
                return vt

            def emit_tile(tt, qt, kt, vt, ot):
                t0 = tt * 128
                # v natural
                psvn = psum_tr.tile([128, V * HD], BF16, tag="tr", name="psvn")
                for v in range(V):
                    nc.tensor.transpose(
                        psvn[:, v * HD : (v + 1) * HD],
                        vt[:, v, t0 : t0 + 128],
                        id_sb[0:64, 0:64],
                    )
                vnat = sm_pool.tile([128, V, HD], BF16, name="vnat")
                nc.scalar.copy(vnat[:], psvn[:])

                # QK -> s^T
                psst = psum_s.tile([44, 512], F32, tag="ss", name="pss")
                pss1 = psst[0:12, :]
                pss2 = psst[32:44, :]
                for j in range(NK):
                    qa = qt[:, j, t0 : t0 + 128]
                    q_b = _bc(qa, [qa.ap[0], [0, 4], qa.ap[-1]])
                    tmpa = tmp_pool.tile([128, 4, 128], BF16, tag="tmpa", name="tmpa")
                    nc.vector.tensor_mul(tmpa[:], q_b, kt[:, 0:4, t0 : t0 + 128])
                    nc.tensor.matmul(
                        pss1,
                        lhsT=ones_sb[:, j, :],
                        rhs=tmpa[:],
                        start=(j == 0),
                        stop=(j == NK - 1),
                        tile_position=(0, 0),
                    )
                    tmpb = tmp_pool.tile([128, 4, 128], BF16, tag="tmpb", name="tmpb")
                    nc.vector.tensor_mul(tmpb[:], q_b, kt[:, 4:8, t0 : t0 + 128])
                    nc.tensor.matmul(
                        pss2,
                        lhsT=ones_sb[:, j, :],
                        rhs=tmpb[:],
                        start=(j == 0),
                        stop=(j == NK - 1),
                        tile_position=(0, 32),
                    )

                # softmax pieces (E unnormalized; Z reciprocal token-major)
                e = sm_pool.tile([12, V * 128], BF16, name="e")
                nc.scalar.activation(e[:, 0:512], pss1, EXP, scale=SCALE)
                nc.scalar.activation(e[:, 512:1024], pss2, EXP, scale=SCALE)
                z = sm_pool.tile([12, 128], F32, name="z")
                ea = e[:]
                e_sw = _bc(ea, [ea.ap[0], [1, 128], [128, V]])
                nc.vector.tensor_reduce(
                    z[:], e_sw, axis=mybir.AxisListType.X, op=mybir.AluOpType.add
                )
                pszn = psum_tr.tile([128, H], F32, tag="tr", name="pszn")
                nc.tensor.transpose(pszn[:], z[:], idf_sb[:])
                rznat = sm_pool.tile([128, H], F32, name="rznat")
                nc.vector.reciprocal(rznat[:], pszn[:])

                # AV: expand P across head columns on PE so the muls pack 2x
                ov = [
                    av_pool.tile([128, H * HD], BF16, tag=f"ov{v}", name=f"ov{v}")
                    for v in range(V)
                ]
                for v in range(V):
                    psxp = psum_tr.tile([128, C], BF16, tag="tr", name="psxp")
                    nc.tensor.transpose(
                        psxp[:, 0:384], e[:, v * 128 : (v + 1) * 128], repl_sb[:, 0:384]
                    )
                    nc.tensor.transpose(
                        psxp[:, 384:768],
                        e[:, v * 128 : (v + 1) * 128],
                        repl_sb[:, 384:768],
                    )
                    pexp = sm_pool.tile([128, C], BF16, tag="pexp", name="pexp")
                    nc.scalar.copy(pexp[:], psxp[:])
                    va = vnat[:, v, :]
                    vn_b = _bc(va, [va.ap[0], [0, H], va.ap[-1]])
                    nc.vector.tensor_mul(ov[v][:], vn_b, pexp[:])
                for a, b2 in ((0, 1), (2, 3), (4, 5), (6, 7), (0, 2), (4, 6), (0, 4)):
                    nc.vector.tensor_add(ov[a][:], ov[a][:], ov[b2][:])
                rza = rznat[:]
                rz_b = _bc(rza, [rza.ap[0], [1, H], [0, HD]])
                nc.vector.tensor_mul(ov[0][:], ov[0][:], rz_b)

                # o^T
                psoa = psum_tr.tile([128, 512], BF16, tag="tr", name="psoa")
                for j in range(4):
                    nc.tensor.transpose(
                        psoa[:, j * 128 : (j + 1) * 128],
                        ov[0][:, j * 128 : (j + 1) * 128],
                        id_sb[:],
                    )
                psob = psum_tr.tile([128, 256], BF16, tag="tr", name="psob")
                for j in range(4, NK):
                    nc.tensor.transpose(
                        psob[:, (j - 4) * 128 : (j - 3) * 128],
                        ov[0][:, j * 128 : (j + 1) * 128],
                        id_sb[:],
                    )
                oa = psoa[:]
                nc.scalar.copy(
                    ot[:, 0:4, t0 : t0 + 128], _bc(oa, [oa.ap[0], [128, 4], [1, 128]])
                )
                ob = psob[:]
                nc.scalar.copy(
                    ot[:, 4:6, t0 : t0 + 128], _bc(ob, [ob.ap[0], [128, 2], [1, 128]])
                )

            def emit_projout(h2, ot, ms=None):
                T0 = h2 * HALF
                for m in (range(NK) if ms is None else ms):
                    pso2 = psum_mm.tile([128, HALF], F32, tag="mm", name="pso2")
                    for k in range(NK):
                        nc.tensor.matmul(
                            pso2[:],
                            lhsT=wp_sb[:, k, m * 128 : (m + 1) * 128],
                            rhs=ot[:, k, :],
                            start=(k == 0),
                            stop=(k == NK - 1),
                        )
                    o2 = out_pool.tile([128, HALF], BF16, name="o2")
                    nc.scalar.activation(
                        o2[:], pso2[:], IDENT, bias=bp_sb[:, m : m + 1], scale=1.0
                    )
                    nc.gpsimd.dma_start(
                        out=outt[m * 128 : (m + 1) * 128, T0 : T0 + HALF], in_=o2[:]
                    )

            # ---- software-pipelined schedule over the two halves
            vpt0 = emit_transposes(0, vs=[0, 1, 2, 3])
            nc.sync.dma_start(
                out=xt_sb[:, :, 0:HALF], in_=xw[0:HALF, :], transpose=True
            )
            nc.sync.dma_start(out=wq_sb[:], in_=wq.rearrange("(j p) o -> p j o", p=128))
            emit_transposes(0, vpt=vpt0, vs=[4, 5, 6, 7])
            emit_late_consts()
            qt0 = acts.tile([128, NK, HALF], BF16, tag="qt", name="qt0")
            kt0 = acts.tile([128, V, HALF], BF16, tag="kt", name="kt0")
            emit_k_group(kt0, vpt0, 0)
            emit_q_chunks(qt0, 0, [0])
            emit_k_group(kt0, vpt0, 4)
            emit_q_chunks(qt0, 0, [1, 2, 3, 4, 5])
            vt0 = emit_v(0, vpt0)
            vpt1 = emit_transposes(1)
            ot0 = acts1.tile([128, NK, HALF], BF16, tag="ot", name="ot0")
            qt1 = kt1 = vt1 = None
            for tt in range(4):
                emit_tile(tt, qt0, kt0, vt0, ot0)
                if tt == 0:
                    qt1 = emit_q(1)
                elif tt == 1:
                    kt1 = emit_k(1, vpt1)
                elif tt == 2:
                    vt1 = emit_v(1, vpt1)
            ot1 = acts1.tile([128, NK, HALF], BF16, tag="ot", name="ot1")
            emit_tile(0, qt1, kt1, vt1, ot1)
            emit_projout(0, ot0, [0, 1])
            emit_tile(1, qt1, kt1, vt1, ot1)
            emit_projout(0, ot0, [2, 3])
            emit_tile(2, qt1, kt1, vt1, ot1)
            emit_projout(0, ot0, [4, 5])
            emit_tile(3, qt1, kt1, vt1, ot1)
            emit_projout(1, ot1)

    _split_multi_waits(nc)
    return nc


_NC = None


def _get_nc():
    global _NC
    if _NC is None:
        _NC = build_kernel()
    return _NC


def _host_inputs(x, variants_patches, Wq, Wkv, Wproj, bproj):
    wq_t = np.ascontiguousarray(np.asarray(Wq, dtype=np.float32).T).astype(BF)
    wk = np.asarray(Wkv, dtype=np.float32)[:HD]
    wv_ = np.asarray(Wkv, dtype=np.float32)[HD:]
    wkk = np.ascontiguousarray(np.concatenate([wk, wk], axis=0).T).astype(BF)
    wv_t = np.ascontiguousarray(wv_.T).astype(BF)
    wp_t = np.ascontiguousarray(np.asarray(Wproj, dtype=np.float32).T).astype(BF)
    bp = np.asarray(bproj, dtype=np.float32).reshape(C, 1)
    ones = np.zeros((C, H), dtype=np.float32)
    for c in range(C):
        ones[c, c // HD] = 1.0
    ones = ones.astype(BF)
    ident = np.eye(128, dtype=np.float32).astype(BF)
    identf_np = np.eye(12, dtype=np.float32)
    repl_np = np.zeros((H, C), dtype=np.float32)
    for c in range(C):
        repl_np[c // HD, c] = 1.0
    repl_np = repl_np.astype(BF)

    x = np.asarray(x, dtype=np.float32)
    vpn = np.asarray(variants_patches, dtype=np.float32)
    in_maps = []
    for b in range(B):
        in_maps.append(
            {
                "xw": x[b].astype(BF),
                "vp": np.ascontiguousarray(vpn[:, b]).astype(BF),
                "wq": wq_t,
                "wkk": wkk,
                "wv": wv_t,
                "wp": wp_t,
                "bp": bp,
                "ones": ones,
                "ident": ident,
                "identf": identf_np,
                "repl": repl_np,
            }
        )
    return in_maps


def run(inputs, trace=False):
    nc = _get_nc()
    in_maps = _host_inputs(
        inputs["x"],
        inputs["variants_patches"],
        inputs["Wq"],
        inputs["Wkv"],
        inputs["Wproj"],
        inputs["bproj"],
    )
    res = run_bass_kernel_spmd(nc, in_maps, core_ids=list(range(8)), trace=trace)
    out = np.stack(
        [np.asarray(res.results[b]["outt"]).T for b in range(B)], axis=0
    ).astype(np.float32)
    return out, res


def kernel(**inputs) -> np.ndarray:
    out, _ = run(inputs, trace=False)
    return out


if __name__ == "__main__":
    rng = np.random.default_rng(0)
    ins = {
        "x": rng.standard_normal((B, N, C)).astype(np.float32),
        "variants_patches": rng.standard_normal((V, B, N, C)).astype(np.float32),
        "Wq": (rng.standard_normal((C, C)) * 0.02).astype(np.float32),
        "Wkv": (rng.standard_normal((2 * HD, C)) * 0.02).astype(np.float32),
        "Wproj": (rng.standard_normal((C, C)) * 0.02).astype(np.float32),
        "bproj": np.zeros((C,), dtype=np.float32),
        "num_layer": 0,
    }
    out = kernel(**ins)
    print("kernel ran, out shape", out.shape)



# revision 5
# speedup vs baseline: 1.0623x; 1.0623x over previous
"""Trainium2 Bass kernel for nn_AttentionModified (MQA-over-variants attention).

Strategy: data-parallel over B across 8 NeuronCores (no collectives — each
batch's output depends only on that batch's inputs).

Per-core pipeline (bf16 compute, f32 PSUM accumulation):
  - activations enter transposed via xbar transpose-DMA (contraction dim on
    partitions); weights pre-transposed on host
  - q^T/k^T/v^T via weight-stationary matmuls (K/V share one LDWEIGHTS across
    3 variants via parallel PSUM accumulation chains)
  - QK logits: broadcast-AP vector multiply (q repeated over variants), then a
    block-ones matmul reduces the 64-wide head groups -> s^T (12 heads, v*t)
  - softmax over V=8 without max-subtraction (logits ~N(0,0.3): exp can't
    overflow); Z via strided segmented reduce; normalization deferred to after
    the AV sum (reciprocal done token-major where it is 8x cheaper)
  - AV combine: broadcast-AP vector mults + pairwise adds
  - output projection; bias fused into PSUM eviction; output written
    transposed, host transposes back
Emission order software-pipelines the two 512-token halves so PE projection
work for half h+1 fills the gaps in the DVE-bound attention phase of half h.
"""
import sys

sys.path.insert(0, "/opt/trn_rl_repo")

import numpy as np
import ml_dtypes

import concourse.bass as bass
import concourse.mybir as mybir
import concourse.tile as tile
from concourse.bass_utils import run_bass_kernel_spmd

BF16 = mybir.dt.bfloat16
F32 = mybir.dt.float32
BF = ml_dtypes.bfloat16

V, B, N, C, H = 8, 8, 1024, 768, 12
HD = C // H  # 64
NK = C // 128  # 6 contraction chunks
HALF = 512
SCALE = HD ** -0.5


def _split_multi_waits(nc):
    """This container's walrus accepts only one sync-wait per instruction;
    hoist extra waits onto same-engine NoOps inserted just before."""
    for f in nc.m.functions:
        for bb in f.blocks:
            new = []
            for inst in bb.instructions:
                si = inst.sync_info
                waits = list(si.on_wait) if (si and si.on_wait) else []
                if len(waits) > 1:
                    for i, w in enumerate(waits[:-1]):
                        nop = mybir.InstNoOp(name=f"{inst.name}-wsplit{i}")
                        nop.engine = inst.engine
                        nop.sync_info = mybir.SyncInfo(on_wait=[w], on_update=[])
                        new.append(nop)
                    si.on_wait = [waits[-1]]
                new.append(inst)
            bb.instructions[:] = new


def _bc(a, dims):
    """Rebuild AP `a` with an explicit dim list (partition dim first)."""
    return bass.AP(tensor=a.tensor, offset=a.offset, ap=dims)


def build_kernel():
    nc = bass.Bass("TRN2", target_bir_lowering=False, debug=False, num_devices=8)

    xw = nc.dram_tensor("xw", [C, N], BF16, kind="ExternalInput").ap()
    vp = nc.dram_tensor("vp", [V, C, N], BF16, kind="ExternalInput").ap()
    wq = nc.dram_tensor("wq", [C, C], BF16, kind="ExternalInput").ap()
    wkk = nc.dram_tensor("wkk", [C, 128], BF16, kind="ExternalInput").ap()
    wv = nc.dram_tensor("wv", [C, HD], BF16, kind="ExternalInput").ap()
    wp = nc.dram_tensor("wp", [C, C], BF16, kind="ExternalInput").ap()
    bp = nc.dram_tensor("bp", [C, 1], F32, kind="ExternalInput").ap()
    ones = nc.dram_tensor("ones", [C, H], BF16, kind="ExternalInput").ap()
    ident = nc.dram_tensor("ident", [128, 128], BF16, kind="ExternalInput").ap()
    identf = nc.dram_tensor("identf", [12, 12], F32, kind="ExternalInput").ap()
    repl = nc.dram_tensor("repl", [12, C], BF16, kind="ExternalInput").ap()
    outt = nc.dram_tensor("outt", [C, N], F32, kind="ExternalOutput").ap()

    EXP = mybir.ActivationFunctionType.Exp
    IDENT = mybir.ActivationFunctionType.Identity

    with tile.TileContext(nc) as tc:
        with (
            tc.tile_pool(name="singles", bufs=1) as singles,
            tc.tile_pool(name="vtp", bufs=2) as vtp_pool,
            tc.tile_pool(name="acts", bufs=2) as acts,
            tc.tile_pool(name="acts1", bufs=1) as acts1,
            tc.tile_pool(name="tmp", bufs=2) as tmp_pool,
            tc.tile_pool(name="sm", bufs=2) as sm_pool,
            tc.tile_pool(name="av", bufs=1) as av_pool,
            tc.tile_pool(name="outp", bufs=1) as out_pool,
            tc.tile_pool(name="psmm", bufs=4, space="PSUM") as psum_mm,
            tc.tile_pool(name="psss", bufs=2, space="PSUM") as psum_s,
            tc.tile_pool(name="pstr", bufs=2, space="PSUM") as psum_tr,
        ):
            # ---- constants (emission order = sync-ring order: small K weight
            # first, then the first transposes, so attention can start early)
            wkk_sb = singles.tile([128, NK, 128], BF16)
            nc.sync.dma_start(out=wkk_sb[:], in_=wkk.rearrange("(j p) o -> p j o", p=128))
            ones_sb = singles.tile([128, NK, H], BF16)
            nc.sync.dma_start(out=ones_sb[:], in_=ones.rearrange("(j p) o -> p j o", p=128))
            id_sb = singles.tile([128, 128], BF16)
            nc.sync.dma_start(out=id_sb[:], in_=ident)
            idf_sb = singles.tile([12, 12], F32)
            nc.sync.dma_start(out=idf_sb[:], in_=identf)
            repl_sb = singles.tile([12, C], BF16)
            nc.sync.dma_start(out=repl_sb[:], in_=repl)
            wq_sb = singles.tile([128, NK, C], BF16)
            wv_sb = singles.tile([128, NK, HD], BF16)
            wp_sb = singles.tile([128, NK, C], BF16)
            bp_sb = singles.tile([128, NK], F32)
            xt_sb = singles.tile([128, NK, N], BF16)


            def emit_late_consts():
                nc.sync.dma_start(
                    out=xt_sb[:, :, HALF:N],
                    in_=xw.rearrange("(j p) n -> p j n", p=128)[:, :, HALF:N],
                )
                nc.sync.dma_start(out=wv_sb[:], in_=wv.rearrange("(j p) o -> p j o", p=128))
                nc.sync.dma_start(out=wp_sb[:], in_=wp.rearrange("(j p) o -> p j o", p=128))
                nc.sync.dma_start(out=bp_sb[:], in_=bp.rearrange("(j p) 1 -> p j", p=128))

            def emit_transposes(h2, vpt=None, vs=None):
                T0 = h2 * HALF
                if vpt is None:
                    vpt = vtp_pool.tile([128, NK, V, HALF], BF16, tag="vpt", name="vpt")
                vpr = vp.rearrange("v (j p) n -> p j v n", p=128)
                for v in (range(V) if vs is None else vs):
                    nc.sync.dma_start(
                        out=vpt[:, :, v, :],
                        in_=vpr[:, :, v, T0 : T0 + HALF],
                    )
                return vpt

            def emit_q_chunks(qt, h2, ms):
                T0 = h2 * HALF
                for m in ms:
                    psq = psum_mm.tile([128, HALF], F32, tag="mm", name="psq")
                    for k in range(NK):
                        nc.tensor.matmul(
                            psq[:],
                            lhsT=wq_sb[:, k, m * 128 : (m + 1) * 128],
                            rhs=xt_sb[:, k, T0 : T0 + HALF],
                            start=(k == 0),
                            stop=(k == NK - 1),
                        )
                    nc.scalar.copy(qt[:, m, :], psq[:])

            def emit_q(h2):
                qt = acts.tile([128, NK, HALF], BF16, tag="qt", name="qt")
                emit_q_chunks(qt, h2, range(NK))
                return qt

            def emit_k_group(kt, vpt, g0):
                for _ in (0,):
                    gn = 4
                    psks = [
                        psum_mm.tile([128, HALF], F32, tag="mm", name=f"psk{i}")
                        for i in range(gn)
                    ]
                    for k in range(NK):
                        for i in range(gn):
                            nc.tensor.matmul(
                                psks[i][:],
                                lhsT=wkk_sb[:, k, :],
                                rhs=vpt[:, k, g0 + i, :],
                                start=(k == 0),
                                stop=(k == NK - 1),
                            )
                    for i in range(gn):
                        nc.scalar.copy(kt[:, g0 + i, :], psks[i][:])

            def emit_k(h2, vpt):
                kt = acts.tile([128, V, HALF], BF16, tag="kt", name="kt")
                emit_k_group(kt, vpt, 0)
                emit_k_group(kt, vpt, 4)
                return kt

            def emit_v(h2, vpt):
                vt = acts.tile([64, V, HALF], BF16, name="vt")
                for g0 in (0, 4):
                    gn = 4
                    psvs = [
                        psum_mm.tile([64, HALF], F32, tag="mm", name=f"psv{i}")
                        for i in range(gn)
                    ]
                    for k in range(NK):
                        for i in range(gn):
                            nc.tensor.matmul(
                                psvs[i][:],
                                lhsT=wv_sb[:, k, :],
                                rhs=vpt[:, k, g0 + i, :],
                                start=(k == 0),
                                stop=(k == NK - 1),
                            )
                    for i in range(gn):
                        nc.scalar.copy(vt[:, g0 + i, :], psvs[i][:])
                return vt

            def emit_tile(tt, qt, kt, vt, ot):
                t0 = tt * 128
                # v natural
                psvn = psum_tr.tile([128, V * HD], BF16, tag="tr", name="psvn")
                for v in range(V):
                    nc.tensor.transpose(
                        psvn[:, v * HD : (v + 1) * HD],
                        vt[:, v, t0 : t0 + 128],
                        id_sb[0:64, 0:64],
                    )
                vnat = sm_pool.tile([128, V, HD], BF16, name="vnat")
                nc.scalar.copy(vnat[:], psvn[:])

                # QK -> s^T
                psst = psum_s.tile([44, 512], F32, tag="ss", name="pss")
                pss1 = psst[0:12, :]
                pss2 = psst[32:44, :]
                for j in range(NK):
                    qa = qt[:, j, t0 : t0 + 128]
                    q_b = _bc(qa, [qa.ap[0], [0, 4], qa.ap[-1]])
                    tmpa = tmp_pool.tile([128, 4, 128], BF16, tag="tmpa", name="tmpa")
                    nc.vector.tensor_mul(tmpa[:], q_b, kt[:, 0:4, t0 : t0 + 128])
                    nc.tensor.matmul(
                        pss1,
                        lhsT=ones_sb[:, j, :],
                        rhs=tmpa[:],
                        start=(j == 0),
                        stop=(j == NK - 1),
                        tile_position=(0, 0),
                    )
                    tmpb = tmp_pool.tile([128, 4, 128], BF16, tag="tmpb", name="tmpb")
                    nc.vector.tensor_mul(tmpb[:], q_b, kt[:, 4:8, t0 : t0 + 128])
                    nc.tensor.matmul(
                        pss2,
                        lhsT=ones_sb[:, j, :],
                        rhs=tmpb[:],
                        start=(j == 0),
                        stop=(j == NK - 1),
                        tile_position=(0, 32),
                    )

                # softmax pieces (E unnormalized; Z reciprocal token-major)
                e = sm_pool.tile([12, V * 128], BF16, name="e")
                nc.scalar.activation(e[:, 0:512], pss1, EXP, scale=SCALE)
                nc.scalar.activation(e[:, 512:1024], pss2, EXP, scale=SCALE)
                z = sm_pool.tile([12, 128], F32, name="z")
                ea = e[:]
                e_sw = _bc(ea, [ea.ap[0], [1, 128], [128, V]])
                nc.vector.tensor_reduce(
                    z[:], e_sw, axis=mybir.AxisListType.X, op=mybir.AluOpType.add
                )
                pszn = psum_tr.tile([128, H], F32, tag="tr", name="pszn")
                nc.tensor.transpose(pszn[:], z[:], idf_sb[:])
                rznat = sm_pool.tile([128, H], F32, name="rznat")
                nc.vector.reciprocal(rznat[:], pszn[:])

                # AV: expand P across head columns on PE so the muls pack 2x
                ov = [
                    av_pool.tile([128, H * HD], BF16, tag=f"ov{v}", name=f"ov{v}")
                    for v in range(V)
                ]
                for v in range(V):
                    psxp = psum_tr.tile([128, C], BF16, tag="tr", name="psxp")
                    nc.tensor.transpose(
                        psxp[:, 0:384], e[:, v * 128 : (v + 1) * 128], repl_sb[:, 0:384]
                    )
                    nc.tensor.transpose(
                        psxp[:, 384:768],
                        e[:, v * 128 : (v + 1) * 128],
                        repl_sb[:, 384:768],
                    )
                    pexp = sm_pool.tile([128, C], BF16, tag="pexp", name="pexp")
                    nc.scalar.copy(pexp[:], psxp[:])
                    va = vnat[:, v, :]
                    vn_b = _bc(va, [va.ap[0], [0, H], va.ap[-1]])
                    nc.vector.tensor_mul(ov[v][:], vn_b, pexp[:])
                for a, b2 in ((0, 1), (2, 3), (4, 5), (6, 7), (0, 2), (4, 6), (0, 4)):
                    nc.vector.tensor_add(ov[a][:], ov[a][:], ov[b2][:])
                rza = rznat[:]
                rz_b = _bc(rza, [rza.ap[0], [1, H], [0, HD]])
                nc.vector.tensor_mul(ov[0][:], ov[0][:], rz_b)

                # o^T
                psoa = psum_tr.tile([128, 512], BF16, tag="tr", name="psoa")
                for j in range(4):
                    nc.tensor.transpose(
                        psoa[:, j * 128 : (j + 1) * 128],
                        ov[0][:, j * 128 : (j + 1) * 128],
                        id_sb[:],
                    )
                psob = psum_tr.tile([128, 256], BF16, tag="tr", name="psob")
                for j in range(4, NK):
                    nc.tensor.transpose(
                        psob[:, (j - 4) * 128 : (j - 3) * 128],
                        ov[0][:, j * 128 : (j + 1) * 128],
                        id_sb[:],
                    )
                oa = psoa[:]
                nc.scalar.copy(
                    ot[:, 0:4, t0 : t0 + 128], _bc(oa, [oa.ap[0], [128, 4], [1, 128]])
                )
                ob = psob[:]
                nc.scalar.copy(
                    ot[:, 4:6, t0 : t0 + 128], _bc(ob, [ob.ap[0], [128, 2], [1, 128]])
                )

            def emit_projout(h2, ot, ms=None):
                T0 = h2 * HALF
                for m in (range(NK) if ms is None else ms):
                    pso2 = psum_mm.tile([128, HALF], F32, tag="mm", name="pso2")
                    for k in range(NK):
                        nc.tensor.matmul(
                            pso2[:],
                            lhsT=wp_sb[:, k, m * 128 : (m + 1) * 128],
                            rhs=ot[:, k, :],
                            start=(k == 0),
                            stop=(k == NK - 1),
                        )
                    o2 = out_pool.tile([128, HALF], BF16, name="o2")
                    nc.scalar.activation(
                        o2[:], pso2[:], IDENT, bias=bp_sb[:, m : m + 1], scale=1.0
                    )
                    nc.gpsimd.dma_start(
                        out=outt[m * 128 : (m + 1) * 128, T0 : T0 + HALF], in_=o2[:]
                    )

            # ---- software-pipelined schedule over the two halves
            vpt0 = emit_transposes(0, vs=[0, 1, 2, 3])
            nc.sync.dma_start(
                out=xt_sb[:, :, 0:HALF],
                in_=xw.rearrange("(j p) n -> p j n", p=128)[:, :, 0:HALF],
            )
            nc.sync.dma_start(out=wq_sb[:], in_=wq.rearrange("(j p) o -> p j o", p=128))
            emit_transposes(0, vpt=vpt0, vs=[4, 5, 6, 7])
            emit_late_consts()
            qt0 = acts.tile([128, NK, HALF], BF16, tag="qt", name="qt0")
            kt0 = acts.tile([128, V, HALF], BF16, tag="kt", name="kt0")
            emit_k_group(kt0, vpt0, 0)
            emit_q_chunks(qt0, 0, [0])
            emit_k_group(kt0, vpt0, 4)
            emit_q_chunks(qt0, 0, [1, 2, 3, 4, 5])
            vt0 = emit_v(0, vpt0)
            vpt1 = emit_transposes(1)
            ot0 = acts1.tile([128, NK, HALF], BF16, tag="ot", name="ot0")
            qt1 = kt1 = vt1 = None
            for tt in range(4):
                emit_tile(tt, qt0, kt0, vt0, ot0)
                if tt == 0:
                    qt1 = emit_q(1)
                elif tt == 1:
                    kt1 = emit_k(1, vpt1)
                elif tt == 2:
                    vt1 = emit_v(1, vpt1)
            ot1 = acts1.tile([128, NK, HALF], BF16, tag="ot", name="ot1")
            emit_tile(0, qt1, kt1, vt1, ot1)
            emit_projout(0, ot0, [0, 1])
            emit_tile(1, qt1, kt1, vt1, ot1)
            emit_projout(0, ot0, [2, 3])
            emit_tile(2, qt1, kt1, vt1, ot1)
            emit_projout(0, ot0, [4, 5])
            emit_tile(3, qt1, kt1, vt1, ot1)
            emit_projout(1, ot1)

    _split_multi_waits(nc)
    return nc


_NC = None


def _get_nc():
    global _NC
    if _NC is None:
        _NC = build_kernel()
    return _NC


def _host_inputs(x, variants_patches, Wq, Wkv, Wproj, bproj):
    wq_t = np.ascontiguousarray(np.asarray(Wq, dtype=np.float32).T).astype(BF)
    wk = np.asarray(Wkv, dtype=np.float32)[:HD]
    wv_ = np.asarray(Wkv, dtype=np.float32)[HD:]
    wkk = np.ascontiguousarray(np.concatenate([wk, wk], axis=0).T).astype(BF)
    wv_t = np.ascontiguousarray(wv_.T).astype(BF)
    wp_t = np.ascontiguousarray(np.asarray(Wproj, dtype=np.float32).T).astype(BF)
    bp = np.asarray(bproj, dtype=np.float32).reshape(C, 1)
    ones = np.zeros((C, H), dtype=np.float32)
    for c in range(C):
        ones[c, c // HD] = 1.0
    ones = ones.astype(BF)
    ident = np.eye(128, dtype=np.float32).astype(BF)
    identf_np = np.eye(12, dtype=np.float32)
    repl_np = np.zeros((H, C), dtype=np.float32)
    for c in range(C):
        repl_np[c // HD, c] = 1.0
    repl_np = repl_np.astype(BF)

    x = np.asarray(x, dtype=np.float32)
    vpn = np.asarray(variants_patches, dtype=np.float32)
    # pre-transpose activations on host: x -> [C, N], vp -> [V, C, N]
    xt = np.ascontiguousarray(x.transpose(0, 2, 1)).astype(BF)  # (B, C, N)
    vpt = np.ascontiguousarray(vpn.transpose(1, 0, 3, 2)).astype(BF)  # (B, V, C, N)
    in_maps = []
    for b in range(B):
        in_maps.append(
            {
                "xw": xt[b],
                "vp": vpt[b],
                "wq": wq_t,
                "wkk": wkk,
                "wv": wv_t,
                "wp": wp_t,
                "bp": bp,
                "ones": ones,
                "ident": ident,
                "identf": identf_np,
                "repl": repl_np,
            }
        )
    return in_maps


def run(inputs, trace=False):
    nc = _get_nc()
    in_maps = _host_inputs(
        inputs["x"],
        inputs["variants_patches"],
        inputs["Wq"],
        inputs["Wkv"],
        inputs["Wproj"],
        inputs["bproj"],
    )
    res = run_bass_kernel_spmd(nc, in_maps, core_ids=list(range(8)), trace=trace)
    out = np.stack(
        [np.asarray(res.results[b]["outt"]).T for b in range(B)], axis=0
    ).astype(np.float32)
    return out, res


def kernel(**inputs) -> np.ndarray:
    out, _ = run(inputs, trace=False)
    return out


if __name__ == "__main__":
    rng = np.random.default_rng(0)
    ins = {
        "x": rng.standard_normal((B, N, C)).astype(np.float32),
        "variants_patches": rng.standard_normal((V, B, N, C)).astype(np.float32),
        "Wq": (rng.standard_normal((C, C)) * 0.02).astype(np.float32),
        "Wkv": (rng.standard_normal((2 * HD, C)) * 0.02).astype(np.float32),
        "Wproj": (rng.standard_normal((C, C)) * 0.02).astype(np.float32),
        "bproj": np.zeros((C,), dtype=np.float32),
        "num_layer": 0,
    }
    out = kernel(**ins)
    print("kernel ran, out shape", out.shape)

